# revision 23
# baseline (speedup 1.0000x reference)
"""FCOS/ATSS detection-decode layer as a Bass/Tile kernel on 8 Trainium2 cores.

Data-parallel over the batch dim: 16 images -> 2 per NeuronCore.
Per core the work is [32768 locations x 80 classes] max/argmax (the memory-
bound bulk), plus small elementwise paths for the box decode and confidence.

Layout per core: partition p (0..127) owns locations [256p, 256p+256).
cls tiles are [128, G, 80] with G locations per partition per tile, so every
DMA moves 320*G contiguous bytes per partition.

Engine split:
  - input DMAs on the SP (sync) HWDGE queue, output DMAs on the ACT queue
    (keeps slow producers from head-of-line-blocking the cls input stream).
  - DVE: per-group max_index (argmax over 80) + batched segmented reduce_max.
  - GPSIMD: box-decode elementwise chain, all on contiguous [P,512] tiles
    (strided Q7 access is pathologically slow).
  - ACT: exp/ln for box decode and confidences (single table set).
"""

import numpy as np

import bass_rust
import concourse.bass as bass
import concourse.mybir as mybir
import concourse.tile as tile
from concourse.bass import AP
from concourse.bass_utils import run_bass_kernel_spmd
from concourse.vector_clock import ScopedClock

# ---------------------------------------------------------------------------
# Patch: this walrus build rejects >1 sync-wait per instruction. Spread the
# Tile tail-drain waits across a chain of SP nops, one wait each.
# ---------------------------------------------------------------------------


def _drain_and_barrier(self, tick_clock, wait_clock):
    nc = self.nc
    drain_inst = nc.sync.drain()
    wait_clock.add_sem_waits(
        drain_inst.ins, ScopedClock({None: tick_clock.global_clock})
    )
    si = drain_inst.ins.sync_info
    waits = list(si.on_wait) if si is not None else []
    if len(waits) > 1:
        drain_inst.ins.sync_info = bass_rust.SyncInfo(
            on_wait=waits[:1], on_update=list(si.on_update)
        )
        for w in waits[1:]:
            nop = nc.sync.nop()
            nop.ins.sync_info = bass_rust.SyncInfo(on_wait=[w], on_update=[])
    nc.all_engine_barrier()
    assert self.sems is not None
    popped = nc._tile_sem_poison_stack.pop()
    assert popped is self._sem_poison
    nc.clear_and_free_semaphores(list(self.sems.allocated().values()))
    nc.all_engine_barrier()


tile.TileContext._drain_and_barrier = _drain_and_barrier

_nop_counter = [0]


def _split_excess_waits(nc):
    """Move excess sync-waits onto injected same-engine nops placed
    immediately before the instruction (walrus allows 1 wait/instruction)."""
    for fn in nc.m.functions:
        for bb in fn.blocks:
            il = bb.instructions
            i = 0
            while i < len(il):
                inst = il[i]
                si = inst.sync_info
                if si is None:
                    i += 1
                    continue
                waits = list(si.on_wait)
                if len(waits) <= 1:
                    i += 1
                    continue
                inst.sync_info = bass_rust.SyncInfo(
                    on_wait=waits[:1], on_update=list(si.on_update)
                )
                pos = i
                for w in waits[1:]:
                    _nop_counter[0] += 1
                    nop = bass_rust.InstNoOp(
                        name=f"wsplit-{_nop_counter[0]}", ins=[], outs=[]
                    )
                    nop.engine = inst.engine
                    nop.sync_info = bass_rust.SyncInfo(on_wait=[w], on_update=[])
                    il.insert(pos, nop)
                    pos += 1
                    i += 1
                i += 1


# ---------------------------------------------------------------------------
# Problem constants (hardcoded per the task contract).
# ---------------------------------------------------------------------------
N_CORES = 8
NB, NH, NW, NCLS = 16, 128, 128, 80
STRIDE = 8
B_PER_CORE = NB // N_CORES           # 2
L = B_PER_CORE * NH * NW             # 32768 locations per core
P = 128                              # partitions
NJ = L // P                          # 256 locations per partition
G = 32                               # locations per partition per cls tile
NT = NJ // G                         # cls tiles per core

F32 = mybir.dt.float32
U32 = mybir.dt.uint32
AX = mybir.AxisListType
ALU = mybir.AluOpType
ACTF = mybir.ActivationFunctionType


def _bcast8(col_ap: AP) -> AP:
    """[P,1] column AP -> [P,8] free-stride-0 broadcast AP."""
    l = [list(d) for d in col_ap.ap]
    return AP(col_ap.tensor, col_ap.offset, [l[0], [0, 8]])


def _build_program(img_h: int, img_w: int):
    nc = bass.Bass()

    cls_in = nc.dram_tensor("cls", [P, NJ, NCLS], F32, kind="ExternalInput")
    ltrb_in = nc.dram_tensor("t_ltrb", [P, NJ * 4], F32, kind="ExternalInput")
    conf_in = nc.dram_tensor("conf", [P, NJ], F32, kind="ExternalInput")
    xywh_out = nc.dram_tensor("xywh", [P, NJ * 4], F32, kind="ExternalOutput")
    idx_out = nc.dram_tensor("idx", [P, NJ], U32, kind="ExternalOutput")
    confs_out = nc.dram_tensor("confs", [P, NJ], F32, kind="ExternalOutput")

    with tile.TileContext(nc) as tc:
        with (
            tc.tile_pool(name="cls", bufs=8) as cls_pool,
            tc.tile_pool(name="stage", bufs=1) as stage_pool,
            tc.tile_pool(name="small", bufs=2) as small_pool,
        ):
            # ---------------- persistent staging ----------------
            mstage = stage_pool.tile([P, NJ], F32)        # per-location max
            idx8all = stage_pool.tile([P, NJ * 8], U32)   # max_index rows
            idxstage = stage_pool.tile([P, NJ], U32)

            # ---------------- cls max + argmax tiles (DVE) ----------------
            for t in range(NT):
                x = cls_pool.tile([P, G * NCLS], F32)
                nc.sync.dma_start(out=x[:], in_=cls_in.ap()[:, t * G:(t + 1) * G, :])
                x3 = x[:].rearrange("p (g c) -> p g c", c=NCLS)
                mslice = mstage[:, t * G:(t + 1) * G]
                nc.vector.reduce_max(mslice, x3, axis=AX.X)
                for g in range(G):
                    col = t * G + g
                    nc.vector.max_index(
                        out=idx8all[:, col * 8:(col + 1) * 8],
                        in_max=_bcast8(mstage[:, col:col + 1]),
                        in_values=x3[:, g, :],
                    )
                if t in (NT // 2 - 1, NT - 1):
                    # gather slot-0 of finished max_index rows (overlapped)
                    h0 = 0 if t == NT // 2 - 1 else NJ // 2
                    hv = idx8all[:, h0 * 8:(h0 + NJ // 2) * 8].rearrange(
                        "p (j k) -> p j k", k=8)
                    nc.vector.tensor_copy(
                        out=idxstage[:, h0:h0 + NJ // 2], in_=hv[:, :, 0])

            nc.scalar.dma_start(out=idx_out.ap(), in_=idxstage[:])

            # ---------------- anchors via iota (interleaved x,y) ----------
            axy = stage_pool.tile([P, NJ * 2], F32)
            ax_view = AP(axy[:].tensor, axy[:].offset,
                         [list(axy[:].ap[0]), [2 * 128, 2], [2, 128]])
            nc.gpsimd.iota(ax_view, pattern=[[0, 2], [8, 128]], base=4,
                           channel_multiplier=0,
                           allow_small_or_imprecise_dtypes=True)
            ay_view = AP(axy[:].tensor, axy[:].offset + 1,
                         [list(axy[:].ap[0]), [2 * 128, 2], [2, 128]])
            nc.gpsimd.iota(ay_view, pattern=[[8, 2], [0, 128]], base=4,
                           channel_multiplier=16,
                           allow_small_or_imprecise_dtypes=True)
            # partitions >= 64 wrapped mod 128: subtract 8*128
            ay_fix = axy[64:128, :].rearrange("p (j two) -> p j two", two=2)[:, :, 1]
            nc.vector.tensor_scalar_add(ay_fix, ay_fix, -1024.0)

            # ---------------- box decode (ltrb) ----------------
            # e_lt/e_rb are contiguous [P, (j,c)] pair-interleaved tiles so
            # every GPSIMD op below runs on stride-1 data.
            ltrb_t = small_pool.tile([P, NJ * 4], F32)
            nc.sync.dma_start(out=ltrb_t[:], in_=ltrb_in.ap())
            l4 = ltrb_t[:].rearrange("p (j c) -> p j c", c=4)
            e_lt = small_pool.tile([P, NJ * 2], F32)
            e_rb = small_pool.tile([P, NJ * 2], F32)
            nc.scalar.activation(
                e_lt[:].rearrange("p (j c) -> p j c", c=2), l4[:, :, 0:2], ACTF.Exp)
            nc.scalar.activation(
                e_rb[:].rearrange("p (j c) -> p j c", c=2), l4[:, :, 2:4], ACTF.Exp)

            x1y1 = small_pool.tile([P, NJ * 2], F32)
            x2y2 = small_pool.tile([P, NJ * 2], F32)
            s8 = small_pool.tile([P, NJ * 2], F32)

            # x1y1 = clip(anchor - 8*e_lt), x2y2 = clip(anchor + 8*e_rb)
            nc.gpsimd.tensor_scalar(out=s8[:], in0=e_lt[:], scalar1=-8.0,
                                    scalar2=None, op0=ALU.mult)
            nc.gpsimd.tensor_tensor(out=x1y1[:], in0=s8[:], in1=axy[:], op=ALU.add)
            nc.gpsimd.tensor_scalar(out=s8[:], in0=e_rb[:], scalar1=8.0,
                                    scalar2=None, op0=ALU.mult)
            nc.gpsimd.tensor_tensor(out=x2y2[:], in0=s8[:], in1=axy[:], op=ALU.add)

            if img_h == img_w:
                nc.gpsimd.tensor_scalar(
                    out=x1y1[:], in0=x1y1[:], scalar1=0.0, scalar2=float(img_w),
                    op0=ALU.max, op1=ALU.min)
                nc.gpsimd.tensor_scalar(
                    out=x2y2[:], in0=x2y2[:], scalar1=0.0, scalar2=float(img_w),
                    op0=ALU.max, op1=ALU.min)
            else:
                x1y1v = x1y1[:].rearrange("p (j c) -> p j c", c=2)
                x2y2v = x2y2[:].rearrange("p (j c) -> p j c", c=2)
                for ch, lim in ((0, img_w), (1, img_h)):
                    nc.vector.tensor_scalar(
                        out=x1y1v[:, :, ch], in0=x1y1v[:, :, ch],
                        scalar1=0.0, scalar2=float(lim), op0=ALU.max, op1=ALU.min)
                    nc.vector.tensor_scalar(
                        out=x2y2v[:, :, ch], in0=x2y2v[:, :, ch],
                        scalar1=0.0, scalar2=float(lim), op0=ALU.max, op1=ALU.min)

            oxy = small_pool.tile([P, NJ * 4], F32)
            oxy4 = oxy[:].rearrange("p (j c) -> p j c", c=4)
            s_t = small_pool.tile([P, NJ * 2], F32)
            nc.gpsimd.tensor_tensor(out=s_t[:], in0=x1y1[:], in1=x2y2[:],
                                    op=ALU.add)
            nc.scalar.activation(oxy4[:, :, 0:2],
                                 s_t[:].rearrange("p (j c) -> p j c", c=2),
                                 ACTF.Copy, scale=0.5)
            nc.vector.tensor_tensor(
                out=oxy4[:, :, 2:4],
                in0=x2y2[:].rearrange("p (j c) -> p j c", c=2),
                in1=x1y1[:].rearrange("p (j c) -> p j c", c=2),
                op=ALU.subtract)
            nc.scalar.dma_start(out=xywh_out.ap(), in_=oxy[:])

            # ---------------- confidences ----------------
            conf_t = small_pool.tile([P, NJ], F32)
            nc.sync.dma_start(out=conf_t[:], in_=conf_in.ap())
            # w1 = ln(1 + exp(-c)) = -ln(sigmoid(c)); w2 likewise from m
            u_t = small_pool.tile([P, NJ], F32)
            nc.scalar.activation(u_t[:], conf_t[:], ACTF.Exp, scale=-1.0)
            w1_t = small_pool.tile([P, NJ], F32)
            nc.scalar.activation(w1_t[:], u_t[:], ACTF.Ln, bias=1.0)
            v_t = small_pool.tile([P, NJ], F32)
            nc.scalar.activation(v_t[:], mstage[:], ACTF.Exp, scale=-1.0)
            w2_t = small_pool.tile([P, NJ], F32)
            nc.scalar.activation(w2_t[:], v_t[:], ACTF.Ln, bias=1.0)
            ssum = small_pool.tile([P, NJ], F32)
            nc.gpsimd.tensor_tensor(out=ssum[:], in0=w1_t[:], in1=w2_t[:],
                                    op=ALU.add)
            # confs = sqrt(sigmoid(c) * sigmoid(m)) = exp(-0.5*(w1+w2))
            cfs = small_pool.tile([P, NJ], F32)
            nc.scalar.activation(cfs[:], ssum[:], ACTF.Exp, scale=-0.5)
            nc.scalar.dma_start(out=confs_out.ap(), in_=cfs[:])

    _split_excess_waits(nc)
    return nc


_PROGRAM_CACHE: dict = {}


def _get_program(img_h: int, img_w: int):
    key = (img_h, img_w)
    if key not in _PROGRAM_CACHE:
        _PROGRAM_CACHE[key] = _build_program(img_h, img_w)
    return _PROGRAM_CACHE[key]


def kernel(t_ltrb, conf_logits, cls_logits, img_h, img_w, _trace=False):
    img_h = int(img_h)
    img_w = int(img_w)
    t_ltrb = np.asarray(t_ltrb, dtype=np.float32)
    conf_logits = np.asarray(conf_logits, dtype=np.float32)
    cls_logits = np.asarray(cls_logits, dtype=np.float32)

    nc = _get_program(img_h, img_w)

    in_maps = []
    for c in range(N_CORES):
        sl = slice(c * B_PER_CORE, (c + 1) * B_PER_CORE)
        in_maps.append({
            "cls": np.ascontiguousarray(cls_logits[sl]).reshape(P, NJ, NCLS),
            "t_ltrb": np.ascontiguousarray(t_ltrb[sl]).reshape(P, NJ * 4),
            "conf": np.ascontiguousarray(conf_logits[sl]).reshape(P, NJ),
        })

    if _trace:
        import tempfile
        tmpdir = tempfile.mkdtemp(prefix="ntff_kernel_")
        res = run_bass_kernel_spmd(
            nc, in_maps, core_ids=list(range(N_CORES)), trace=True,
            tmpdir=tmpdir)
        kernel._last_tmpdir = tmpdir
    else:
        res = run_bass_kernel_spmd(nc, in_maps, core_ids=list(range(N_CORES)))

    p_xywh = np.empty((NB, NH * NW, 4), np.float32)
    cls_idx = np.empty((NB, NH * NW), np.int32)
    confs = np.empty((NB, NH * NW), np.float32)
    for c in range(N_CORES):
        sl = slice(c * B_PER_CORE, (c + 1) * B_PER_CORE)
        r = res.results[c]
        p_xywh[sl] = r["xywh"].reshape(B_PER_CORE, NH * NW, 4)
        cls_idx[sl] = r["idx"].view(np.int32).reshape(B_PER_CORE, NH * NW)
        confs[sl] = r["confs"].reshape(B_PER_CORE, NH * NW)

    if _trace:
        kernel._last_results = res
    return p_xywh, cls_idx, confs


# revision 25
# speedup vs baseline: 1.0486x; 1.0486x over previous
"""FCOS/ATSS detection-decode layer as a Bass/Tile kernel on 8 Trainium2 cores.

Data-parallel over the batch dim: 16 images -> 2 per NeuronCore.
Per core the work is [32768 locations x 80 classes] max/argmax (the memory-
bound bulk), plus small elementwise paths for the box decode and confidence.

Layout per core: partition p (0..127) owns locations [256p, 256p+256).
cls tiles are [128, G, 80] with G locations per partition per tile, so every
DMA moves 320*G contiguous bytes per partition.

Engine split:
  - input DMAs on the SP (sync) HWDGE queue, output DMAs on the ACT queue
    (keeps slow producers from head-of-line-blocking the cls input stream).
  - DVE: per-group max_index (argmax over 80) + batched segmented reduce_max.
  - GPSIMD: box-decode elementwise chain, all on contiguous [P,512] tiles
    (strided Q7 access is pathologically slow).
  - ACT: exp/ln for box decode and confidences (single table set).
"""

import numpy as np

import bass_rust
import concourse.bass as bass
import concourse.mybir as mybir
import concourse.tile as tile
from concourse.bass import AP
from concourse.bass_utils import run_bass_kernel_spmd
from concourse.vector_clock import ScopedClock

# ---------------------------------------------------------------------------
# Patch: this walrus build rejects >1 sync-wait per instruction. Spread the
# Tile tail-drain waits across a chain of SP nops, one wait each.
# ---------------------------------------------------------------------------


def _drain_and_barrier(self, tick_clock, wait_clock):
    nc = self.nc
    drain_inst = nc.sync.drain()
    wait_clock.add_sem_waits(
        drain_inst.ins, ScopedClock({None: tick_clock.global_clock})
    )
    si = drain_inst.ins.sync_info
    waits = list(si.on_wait) if si is not None else []
    if len(waits) > 1:
        drain_inst.ins.sync_info = bass_rust.SyncInfo(
            on_wait=waits[:1], on_update=list(si.on_update)
        )
        for w in waits[1:]:
            nop = nc.sync.nop()
            nop.ins.sync_info = bass_rust.SyncInfo(on_wait=[w], on_update=[])
    nc.all_engine_barrier()
    assert self.sems is not None
    popped = nc._tile_sem_poison_stack.pop()
    assert popped is self._sem_poison
    nc.clear_and_free_semaphores(list(self.sems.allocated().values()))
    nc.all_engine_barrier()


tile.TileContext._drain_and_barrier = _drain_and_barrier

_nop_counter = [0]


def _split_excess_waits(nc):
    """Move excess sync-waits onto injected same-engine nops placed
    immediately before the instruction (walrus allows 1 wait/instruction)."""
    for fn in nc.m.functions:
        for bb in fn.blocks:
            il = bb.instructions
            i = 0
            while i < len(il):
                inst = il[i]
                si = inst.sync_info
                if si is None:
                    i += 1
                    continue
                waits = list(si.on_wait)
                if len(waits) <= 1:
                    i += 1
                    continue
                inst.sync_info = bass_rust.SyncInfo(
                    on_wait=waits[:1], on_update=list(si.on_update)
                )
                pos = i
                for w in waits[1:]:
                    _nop_counter[0] += 1
                    nop = bass_rust.InstNoOp(
                        name=f"wsplit-{_nop_counter[0]}", ins=[], outs=[]
                    )
                    nop.engine = inst.engine
                    nop.sync_info = bass_rust.SyncInfo(on_wait=[w], on_update=[])
                    il.insert(pos, nop)
                    pos += 1
                    i += 1
                i += 1


# ---------------------------------------------------------------------------
# Problem constants (hardcoded per the task contract).
# ---------------------------------------------------------------------------
N_CORES = 8
NB, NH, NW, NCLS = 16, 128, 128, 80
STRIDE = 8
B_PER_CORE = NB // N_CORES           # 2
L = B_PER_CORE * NH * NW             # 32768 locations per core
P = 128                              # partitions
NJ = L // P                          # 256 locations per partition
G = 32                               # locations per partition per cls tile
NT = NJ // G                         # cls tiles per core

F32 = mybir.dt.float32
U32 = mybir.dt.uint32
AX = mybir.AxisListType
ALU = mybir.AluOpType
ACTF = mybir.ActivationFunctionType


def _bcast8(col_ap: AP) -> AP:
    """[P,1] column AP -> [P,8] free-stride-0 broadcast AP."""
    l = [list(d) for d in col_ap.ap]
    return AP(col_ap.tensor, col_ap.offset, [l[0], [0, 8]])


def _build_program(img_h: int, img_w: int):
    nc = bass.Bass()

    cls_in = nc.dram_tensor("cls", [P, NJ, NCLS], F32, kind="ExternalInput")
    ltrb_in = nc.dram_tensor("t_ltrb", [P, NJ * 4], F32, kind="ExternalInput")
    conf_in = nc.dram_tensor("conf", [P, NJ], F32, kind="ExternalInput")
    xywh_out = nc.dram_tensor("xywh", [P, NJ * 4], F32, kind="ExternalOutput")
    idx_out = nc.dram_tensor("idx", [P, NJ], U32, kind="ExternalOutput")
    confs_out = nc.dram_tensor("confs", [P, NJ], F32, kind="ExternalOutput")

    with tile.TileContext(nc) as tc:
        with (
            tc.tile_pool(name="cls", bufs=6) as cls_pool,
            tc.tile_pool(name="stage", bufs=1) as stage_pool,
            tc.tile_pool(name="small", bufs=2) as small_pool,
        ):
            # ---------------- persistent staging ----------------
            mstage = stage_pool.tile([P, NJ], F32)        # per-location max
            idx8all = stage_pool.tile([P, NJ * 8], U32)   # max_index rows
            idxstage = stage_pool.tile([P, NJ], U32)

            # ---------------- cls max + argmax tiles (DVE) ----------------
            for t in range(NT):
                x = cls_pool.tile([P, G * NCLS], F32)
                nc.sync.dma_start(out=x[:], in_=cls_in.ap()[:, t * G:(t + 1) * G, :])
                x3 = x[:].rearrange("p (g c) -> p g c", c=NCLS)
                mslice = mstage[:, t * G:(t + 1) * G]
                nc.vector.reduce_max(mslice, x3, axis=AX.X)
                for g in range(G):
                    col = t * G + g
                    nc.vector.max_index(
                        out=idx8all[:, col * 8:(col + 1) * 8],
                        in_max=_bcast8(mstage[:, col:col + 1]),
                        in_values=x3[:, g, :],
                    )

            # gather slot-0 of every max_index row -> idxstage (GPSIMD)
            idx8v = idx8all[:].rearrange("p (j k) -> p j k", k=8)
            nc.vector.tensor_copy(out=idxstage[:], in_=idx8v[:, :, 0])
            nc.scalar.dma_start(out=idx_out.ap(), in_=idxstage[:])

            # ---------------- anchors via iota (interleaved x,y) ----------
            axy = stage_pool.tile([P, NJ * 2], F32)
            ax_view = AP(axy[:].tensor, axy[:].offset,
                         [list(axy[:].ap[0]), [2 * 128, 2], [2, 128]])
            nc.gpsimd.iota(ax_view, pattern=[[0, 2], [8, 128]], base=4,
                           channel_multiplier=0,
                           allow_small_or_imprecise_dtypes=True)
            ay_view = AP(axy[:].tensor, axy[:].offset + 1,
                         [list(axy[:].ap[0]), [2 * 128, 2], [2, 128]])
            nc.gpsimd.iota(ay_view, pattern=[[8, 2], [0, 128]], base=4,
                           channel_multiplier=16,
                           allow_small_or_imprecise_dtypes=True)
            # partitions >= 64 wrapped mod 128: subtract 8*128
            ay_fix = axy[64:128, :].rearrange("p (j two) -> p j two", two=2)[:, :, 1]
            nc.vector.tensor_scalar_add(ay_fix, ay_fix, -1024.0)

            # ---------------- box decode (ltrb) ----------------
            # e_lt/e_rb are contiguous [P, (j,c)] pair-interleaved tiles so
            # every GPSIMD op below runs on stride-1 data.
            ltrb_t = small_pool.tile([P, NJ * 4], F32)
            nc.sync.dma_start(out=ltrb_t[:], in_=ltrb_in.ap())
            l4 = ltrb_t[:].rearrange("p (j c) -> p j c", c=4)
            e_lt = small_pool.tile([P, NJ * 2], F32)
            e_rb = small_pool.tile([P, NJ * 2], F32)
            nc.scalar.activation(
                e_lt[:].rearrange("p (j c) -> p j c", c=2), l4[:, :, 0:2], ACTF.Exp)
            nc.scalar.activation(
                e_rb[:].rearrange("p (j c) -> p j c", c=2), l4[:, :, 2:4], ACTF.Exp)

            x1y1 = small_pool.tile([P, NJ * 2], F32)
            x2y2 = small_pool.tile([P, NJ * 2], F32)
            s8 = small_pool.tile([P, NJ * 2], F32)

            # x1y1 = clip(anchor - 8*e_lt), x2y2 = clip(anchor + 8*e_rb)
            nc.gpsimd.tensor_scalar(out=s8[:], in0=e_lt[:], scalar1=-8.0,
                                    scalar2=None, op0=ALU.mult)
            nc.gpsimd.tensor_tensor(out=x1y1[:], in0=s8[:], in1=axy[:], op=ALU.add)
            nc.gpsimd.tensor_scalar(out=s8[:], in0=e_rb[:], scalar1=8.0,
                                    scalar2=None, op0=ALU.mult)
            nc.gpsimd.tensor_tensor(out=x2y2[:], in0=s8[:], in1=axy[:], op=ALU.add)

            if img_h == img_w:
                nc.gpsimd.tensor_scalar(
                    out=x1y1[:], in0=x1y1[:], scalar1=0.0, scalar2=float(img_w),
                    op0=ALU.max, op1=ALU.min)
                nc.gpsimd.tensor_scalar(
                    out=x2y2[:], in0=x2y2[:], scalar1=0.0, scalar2=float(img_w),
                    op0=ALU.max, op1=ALU.min)
            else:
                x1y1v = x1y1[:].rearrange("p (j c) -> p j c", c=2)
                x2y2v = x2y2[:].rearrange("p (j c) -> p j c", c=2)
                for ch, lim in ((0, img_w), (1, img_h)):
                    nc.vector.tensor_scalar(
                        out=x1y1v[:, :, ch], in0=x1y1v[:, :, ch],
                        scalar1=0.0, scalar2=float(lim), op0=ALU.max, op1=ALU.min)
                    nc.vector.tensor_scalar(
                        out=x2y2v[:, :, ch], in0=x2y2v[:, :, ch],
                        scalar1=0.0, scalar2=float(lim), op0=ALU.max, op1=ALU.min)

            oxy = small_pool.tile([P, NJ * 4], F32)
            oxy4 = oxy[:].rearrange("p (j c) -> p j c", c=4)
            s_t = small_pool.tile([P, NJ * 2], F32)
            nc.gpsimd.tensor_tensor(out=s_t[:], in0=x1y1[:], in1=x2y2[:],
                                    op=ALU.add)
            nc.scalar.activation(oxy4[:, :, 0:2],
                                 s_t[:].rearrange("p (j c) -> p j c", c=2),
                                 ACTF.Copy, scale=0.5)
            nc.vector.tensor_tensor(
                out=oxy4[:, :, 2:4],
                in0=x2y2[:].rearrange("p (j c) -> p j c", c=2),
                in1=x1y1[:].rearrange("p (j c) -> p j c", c=2),
                op=ALU.subtract)
            nc.scalar.dma_start(out=xywh_out.ap(), in_=oxy[:])

            # ---------------- confidences ----------------
            conf_t = small_pool.tile([P, NJ], F32)
            nc.sync.dma_start(out=conf_t[:], in_=conf_in.ap())
            # w1 = ln(1 + exp(-c)) = -ln(sigmoid(c)); w2 likewise from m
            u_t = small_pool.tile([P, NJ], F32)
            nc.scalar.activation(u_t[:], conf_t[:], ACTF.Exp, scale=-1.0)
            w1_t = small_pool.tile([P, NJ], F32)
            nc.scalar.activation(w1_t[:], u_t[:], ACTF.Ln, bias=1.0)
            v_t = small_pool.tile([P, NJ], F32)
            nc.scalar.activation(v_t[:], mstage[:], ACTF.Exp, scale=-1.0)
            w2_t = small_pool.tile([P, NJ], F32)
            nc.scalar.activation(w2_t[:], v_t[:], ACTF.Ln, bias=1.0)
            ssum = small_pool.tile([P, NJ], F32)
            nc.gpsimd.tensor_tensor(out=ssum[:], in0=w1_t[:], in1=w2_t[:],
                                    op=ALU.add)
            # confs = sqrt(sigmoid(c) * sigmoid(m)) = exp(-0.5*(w1+w2))
            cfs = small_pool.tile([P, NJ], F32)
            nc.scalar.activation(cfs[:], ssum[:], ACTF.Exp, scale=-0.5)
            nc.scalar.dma_start(out=confs_out.ap(), in_=cfs[:])

    _split_excess_waits(nc)
    return nc


_PROGRAM_CACHE: dict = {}


def _get_program(img_h: int, img_w: int):
    key = (img_h, img_w)
    if key not in _PROGRAM_CACHE:
        _PROGRAM_CACHE[key] = _build_program(img_h, img_w)
    return _PROGRAM_CACHE[key]


def kernel(t_ltrb, conf_logits, cls_logits, img_h, img_w, _trace=False):
    img_h = int(img_h)
    img_w = int(img_w)
    t_ltrb = np.asarray(t_ltrb, dtype=np.float32)
    conf_logits = np.asarray(conf_logits, dtype=np.float32)
    cls_logits = np.asarray(cls_logits, dtype=np.float32)

    nc = _get_program(img_h, img_w)

    in_maps = []
    for c in range(N_CORES):
        sl = slice(c * B_PER_CORE, (c + 1) * B_PER_CORE)
        in_maps.append({
            "cls": np.ascontiguousarray(cls_logits[sl]).reshape(P, NJ, NCLS),
            "t_ltrb": np.ascontiguousarray(t_ltrb[sl]).reshape(P, NJ * 4),
            "conf": np.ascontiguousarray(conf_logits[sl]).reshape(P, NJ),
        })

    if _trace:
        import tempfile
        tmpdir = tempfile.mkdtemp(prefix="ntff_kernel_")
        res = run_bass_kernel_spmd(
            nc, in_maps, core_ids=list(range(N_CORES)), trace=True,
            tmpdir=tmpdir)
        kernel._last_tmpdir = tmpdir
    else:
        res = run_bass_kernel_spmd(nc, in_maps, core_ids=list(range(N_CORES)))

    p_xywh = np.empty((NB, NH * NW, 4), np.float32)
    cls_idx = np.empty((NB, NH * NW), np.int32)
    confs = np.empty((NB, NH * NW), np.float32)
    for c in range(N_CORES):
        sl = slice(c * B_PER_CORE, (c + 1) * B_PER_CORE)
        r = res.results[c]
        p_xywh[sl] = r["xywh"].reshape(B_PER_CORE, NH * NW, 4)
        cls_idx[sl] = r["idx"].view(np.int32).reshape(B_PER_CORE, NH * NW)
        confs[sl] = r["confs"].reshape(B_PER_CORE, NH * NW)

    if _trace:
        kernel._last_results = res
    return p_xywh, cls_idx, confs
